# revision 23
# baseline (speedup 1.0000x reference)
"""Trainium2 Bass kernel for BiDAF-style bidirectional attention.

Reference math (per batch b):
    sim[c,q]  = q[q]·wq + c[c]·wc + sum_e wm[e]*question[q,e]*context[c,e]
    c2q[c,:]  = softmax_q(sim[c,:]) @ question          # (C, E)
    q2c[:]    = softmax_c(max_q sim[c,:]) @ context     # (E,)
    out[c,:]  = [context | c2q | context*c2q | context*q2c]

Sharding: pure data parallel over batch (B=16 -> 2 batches per core x 8 cores).

Context tiles (128 rows of C on partitions) are processed in PAIRS so the
small vector-engine ops amortize their fixed per-op cost:
  - PE transposes Xc (fp32) -> XcT psum; the PSUM->SBUF copy rounds to fp32r
  - sim psum (128, 2, 256) accumulates XcT.T @ rhs_aug per tile (fp32r,
    N=256 to hit the single-pass fast path; col 128 carries wc·ctx for the
    q2c stats, cols 129:255 are zero pad)
  - one DVE add applies the q-weighted row to both tiles; 3D-AP reduce_max
    gets both row-maxes in one op
  - exp on the scalar engine emits bf16 attention weights + fp32 row-sums
  - attn^T (bf16 PE transpose) @ question (bf16) -> c2q, row-rescaled by
    1/rowsum during the PSUM->SBUF copy on the scalar engine
  - q2c (second pass, softmax over C of the row maxes): global max/sum via
    PE-transpose + ones-matmul broadcasts; 16 rank-1 fp32 matmuls
  - outputs assemble in a (128, 1024) staging tile per C-tile; cols 0:768
    ship in pass 1, 768:1024 in pass 2 (wide DMA bursts); the elementwise
    products run on GpSimd to keep the vector engine free
"""

import numpy as np

import concourse.bass as bass
import concourse.tile as tile
import concourse.mybir as mybir
from concourse import bacc
from concourse.bass_utils import run_bass_kernel_spmd
from concourse.masks import make_identity

B, C, Q, E = 16, 2048, 128, 256
NCORES = 8
BPC = B // NCORES          # batches per core
NT = C // 128              # context tiles per batch
F32 = mybir.dt.float32
F32R = mybir.dt.float32r
BF16 = mybir.dt.bfloat16


def _body(tc, out_ext, ctx_in, q_in, wq_in, wc_in, wm_in):
    nc = tc.nc
    with (
        tc.tile_pool(name="singles", bufs=1) as singles,
        tc.tile_pool(name="xcpool", bufs=9) as xcp,
        tc.tile_pool(name="qside", bufs=2) as qside,
        tc.tile_pool(name="xbfpool", bufs=9) as xbfp,
        tc.tile_pool(name="work", bufs=4) as work,
        tc.tile_pool(name="statsp", bufs=2) as statsp,
        tc.tile_pool(name="ps_xct", bufs=2, space="PSUM") as ps_xct,
        tc.tile_pool(name="ps_sim", bufs=2, space="PSUM") as ps_sim,
        tc.tile_pool(name="ps_pt", bufs=1, space="PSUM") as ps_pt,
        tc.tile_pool(name="ps_c2q", bufs=1, space="PSUM") as ps_c2q,
        tc.tile_pool(name="ps_q2c", bufs=1, space="PSUM") as ps_q2c,
        tc.tile_pool(name="ps_misc", bufs=1, space="PSUM") as ps_misc,
    ):
        ident = singles.tile([128, 128], F32)
        make_identity(nc, ident)
        ident_bf = singles.tile([128, 128], BF16)
        make_identity(nc, ident_bf)
        ones_r = singles.tile([1, 128], F32)
        nc.vector.memset(ones_r, 1.0)
        ones_c = singles.tile([128, 1], F32)
        nc.vector.memset(ones_c, 1.0)
        # rank-1 params laid out (128 partitions, 2 chunks of E)
        wq_sb = singles.tile([128, 2], F32)
        nc.sync.dma_start(out=wq_sb, in_=wq_in.rearrange("(j p) -> p j", p=128))
        wc_sb = singles.tile([128, 2], F32)
        nc.sync.dma_start(out=wc_sb, in_=wc_in.rearrange("(j p) -> p j", p=128))
        wm_sb = singles.tile([128, 2], F32)
        nc.sync.dma_start(out=wm_sb, in_=wm_in.rearrange("(j p) -> p j", p=128))

        mstats = {}
        groups_all = {}
        for b in range(BPC):
            groups_all[b] = []
            for g in range(NT // 4):
                stg = xcp.tile([128, 4, 4 * E], F32, tag="stg")
                groups_all[b].append(stg)
                nc.sync.dma_start(
                    out=stg[:, :, 0:E],
                    in_=ctx_in[b, g * 512 : (g + 1) * 512, :].rearrange(
                        "(t p) e -> p t e", p=128
                    ),
                )
        for b in range(BPC):
            # ---- phase A: question-side prep -------------------------------
            qm = qside.tile([128, E], F32)
            nc.sync.dma_start(out=qm, in_=q_in[b])
            qmt_ps = ps_xct.tile([128, E], F32, tag="xct")
            for j in range(2):
                nc.tensor.transpose(
                    qmt_ps[:, j * 128 : (j + 1) * 128],
                    qm[:, j * 128 : (j + 1) * 128],
                    ident,
                )
            qmt_sb = qside.tile([128, E], F32)
            nc.vector.tensor_copy(out=qmt_sb, in_=qmt_ps)
            # bf16 copy of the question for the c2q matmul
            qm_bf = qside.tile([128, E], BF16)
            nc.vector.tensor_copy(out=qm_bf, in_=qm)
            # rhs_aug[:, j, 0:128] = wm-chunk * QmT-chunk ; [:, j, 128] = wc-chunk
            # cols 129:256 are zero padding so the fp32r matmul runs at N=256.
            rhs_aug = qside.tile([128, 2, E], F32R)
            for j in range(2):
                nc.vector.tensor_scalar_mul(
                    rhs_aug[:, j, 0:128],
                    qmt_sb[:, j * 128 : (j + 1) * 128],
                    wm_sb[:, j : j + 1],
                )
                nc.vector.tensor_copy(
                    out=rhs_aug[:, j, 128:129], in_=wc_sb[:, j : j + 1]
                )
                # memset can't write f32r; x0.0 DVE op produces rounded zeros
                nc.vector.tensor_scalar_mul(
                    rhs_aug[:, j, 129:256],
                    qmt_sb[:, j * 128 : (j + 1) * 128][:, 0:127],
                    0.0,
                )
            # q_weighted row (1, Q) -> broadcast to 128 partitions, twice wide
            qw_ps = ps_misc.tile([1, 128], F32, tag="misc")
            for j in range(2):
                nc.tensor.matmul(
                    qw_ps,
                    wq_sb[:, j : j + 1],
                    qmt_sb[:, j * 128 : (j + 1) * 128],
                    start=(j == 0),
                    stop=(j == 1),
                )
            qw_row = qside.tile([1, 128], F32)
            nc.vector.tensor_copy(out=qw_row, in_=qw_ps)
            qwb_ps = ps_misc.tile([128, 128], F32, tag="misc")
            nc.tensor.matmul(qwb_ps, ones_r, qw_row, start=True, stop=True)
            qw_bcast2 = qside.tile([128, 2, 128], F32)
            nc.vector.tensor_copy(out=qw_bcast2[:, 0, :], in_=qwb_ps)
            nc.vector.tensor_copy(out=qw_bcast2[:, 1, :], in_=qwb_ps)

            mstat = statsp.tile([128, NT], F32)
            mstats[b] = mstat
            groups = groups_all[b]

            # ---- phase B: pass 1, groups of 4 context tiles ----------------
            for g in range(NT // 4):
                stg = groups[g]
                for h in range(2):
                    sim_ps = ps_sim.tile([128, 2, E], F32, tag="sim")
                    for i in range(2):
                        lane = 2 * h + i
                        xc = stg[:, lane, 0:E]
                        xct_ps = ps_xct.tile([128, E], F32, tag="xct")
                        for j in range(2):
                            nc.tensor.transpose(
                                xct_ps[:, j * 128 : (j + 1) * 128],
                                xc[:, j * 128 : (j + 1) * 128],
                                ident,
                            )
                        xct_sb = work.tile([128, E], F32R, tag="xct_sb")
                        nc.vector.tensor_copy(out=xct_sb, in_=xct_ps)
                        for j in range(2):
                            nc.tensor.matmul(
                                sim_ps[:, i, :],
                                xct_sb[:, j * 128 : (j + 1) * 128],
                                rhs_aug[:, j, :],
                                start=(j == 0),
                                stop=(j == 1),
                            )

                    # add the q-weighted row for both tiles in one DVE op
                    sim_in = work.tile([128, 2, 128], F32, tag="sim_in")
                    nc.vector.tensor_add(sim_in, sim_ps[:, :, 0:128], qw_bcast2)
                    neg_m = work.tile([128, 2], F32, tag="neg_m")
                    nc.vector.reduce_max(
                        out=neg_m,
                        in_=sim_in,
                        axis=mybir.AxisListType.X,
                        negate=True,
                    )
                    # q2c stats: max_q(full sim) = wc·ctx (col 128) + row-max
                    t0 = 4 * g + 2 * h
                    nc.vector.tensor_sub(
                        mstat[:, t0 : t0 + 2], sim_ps[:, :, 128], neg_m
                    )
                    row_sum = work.tile([128, 2], F32, tag="row_sum")
                    recip = work.tile([128, 2], F32, tag="recip")
                    p_sb = work.tile([128, 2, 128], BF16, tag="p_sb")
                    for i in range(2):
                        nc.scalar.activation(
                            out=p_sb[:, i, :],
                            in_=sim_in[:, i, :],
                            func=mybir.ActivationFunctionType.Exp,
                            bias=neg_m[:, i : i + 1],
                            scale=1.0,
                            accum_out=row_sum[:, i : i + 1],
                        )
                    nc.vector.reciprocal(out=recip, in_=row_sum)

                    for i in range(2):
                        lane = 2 * h + i
                        pt_ps = ps_pt.tile([128, 128], BF16, tag="pt")
                        nc.tensor.transpose(pt_ps, p_sb[:, i, :], ident_bf)
                        pt_sb = work.tile([128, 128], BF16, tag="pt_sb")
                        nc.vector.tensor_copy(out=pt_sb, in_=pt_ps)

                        c2q_ps = ps_c2q.tile([128, E], F32, tag="c2q")
                        nc.tensor.matmul(
                            c2q_ps, pt_sb, qm_bf, start=True, stop=True
                        )
                        nc.scalar.activation(
                            out=stg[:, lane, E : 2 * E],
                            in_=c2q_ps,
                            func=mybir.ActivationFunctionType.Copy,
                            scale=recip[:, i : i + 1],
                        )

                # ctx * c2q for all 4 lanes in one op, then one wide store
                nc.gpsimd.tensor_mul(
                    stg[:, :, 2 * E : 3 * E],
                    stg[:, :, 0:E],
                    stg[:, :, E : 2 * E],
                )
                nc.sync.dma_start(
                    out=out_ext[b, g * 512 : (g + 1) * 512, 0 : 3 * E].rearrange(
                        "(t p) f -> p t f", p=128
                    ),
                    in_=stg[:, :, 0 : 3 * E],
                )

        for b in range(BPC):
            mstat = mstats[b]
            groups = groups_all[b]
            # ---- phase C: q2c attention over the context axis --------------
            r1 = statsp.tile([128, 1], F32, tag="r1")
            nc.vector.reduce_max(out=r1, in_=mstat, axis=mybir.AxisListType.X)
            r1t_ps = ps_misc.tile([1, 128], F32, tag="misc")
            nc.tensor.transpose(r1t_ps, r1, ident)
            neg_gmax = statsp.tile([1, 1], F32, tag="neg_gmax")
            nc.vector.reduce_max(
                out=neg_gmax, in_=r1t_ps, axis=mybir.AxisListType.X, negate=True
            )
            ngb_ps = ps_misc.tile([128, 1], F32, tag="misc")
            nc.tensor.matmul(ngb_ps, ones_r, neg_gmax, start=True, stop=True)
            ngb_sb = statsp.tile([128, 1], F32, tag="ngb_sb")
            nc.vector.tensor_copy(out=ngb_sb, in_=ngb_ps)

            e_sb = statsp.tile([128, NT], BF16, tag="e_sb")
            s_col = statsp.tile([128, 1], F32, tag="s_col")
            nc.scalar.activation(
                out=e_sb,
                in_=mstat,
                func=mybir.ActivationFunctionType.Exp,
                bias=ngb_sb,
                scale=1.0,
                accum_out=s_col,
            )
            tot_ps = ps_misc.tile([1, 1], F32, tag="misc")
            nc.tensor.matmul(tot_ps, s_col, ones_c, start=True, stop=True)
            rt_sb = statsp.tile([1, 1], F32, tag="rt_sb")
            nc.vector.reciprocal(out=rt_sb, in_=tot_ps)

            # bf16 context copies (cold-clock bf16 matmuls are 4x faster
            # than fp32 ones); alternate engines so the casts parallelize
            xcbfs = []
            for g in range(NT // 4):
                xcbf = xbfp.tile([128, 4, E], BF16, tag="xcbf")
                xcbfs.append(xcbf)
                eng = (
                    nc.gpsimd if (b == BPC - 1 and g % 2 == 1) else nc.vector
                )
                eng.tensor_copy(out=xcbf, in_=groups[g][:, :, 0:E])
            q2c_ps = ps_q2c.tile([1, E], F32, tag="q2c")
            for t in range(NT):
                nc.tensor.matmul(
                    q2c_ps,
                    e_sb[:, t : t + 1],
                    xcbfs[t // 4][:, t % 4, :],
                    start=(t == 0),
                    stop=(t == NT - 1),
                )
            q2c_sb = statsp.tile([1, E], F32, tag="q2c_sb")
            nc.scalar.activation(
                out=q2c_sb,
                in_=q2c_ps,
                func=mybir.ActivationFunctionType.Copy,
                scale=rt_sb,
            )
            q2cb_ps = ps_misc.tile([128, E], F32, tag="misc")
            nc.tensor.matmul(q2cb_ps, ones_r, q2c_sb, start=True, stop=True)
            q2cb_sb = statsp.tile([128, 4, E], F32, tag="q2cb_sb")
            for lane in range(4):
                nc.vector.tensor_copy(out=q2cb_sb[:, lane, :], in_=q2cb_ps)

            # ---- phase D: pass 2 -> context * q2c ---------------------------
            for g in range(NT // 4):
                stg = groups[g]
                eng = (
                    nc.gpsimd if (b == BPC - 1 and g % 2 == 1) else nc.vector
                )
                eng.tensor_mul(
                    stg[:, :, 3 * E : 4 * E], stg[:, :, 0:E], q2cb_sb
                )
                nc.sync.dma_start(
                    out=out_ext[
                        b, g * 512 : (g + 1) * 512, 3 * E : 4 * E
                    ].rearrange("(t p) f -> p t f", p=128),
                    in_=stg[:, :, 3 * E : 4 * E],
                )


_NC_CACHE = None


def _build():
    global _NC_CACHE
    if _NC_CACHE is not None:
        return _NC_CACHE
    nc = bacc.Bacc(
        "TRN2", target_bir_lowering=False, debug=False, num_devices=NCORES
    )
    ctx_in = nc.dram_tensor("context", [BPC, C, E], F32, kind="ExternalInput").ap()
    q_in = nc.dram_tensor("question", [BPC, Q, E], F32, kind="ExternalInput").ap()
    wq_in = nc.dram_tensor("w_question", [E], F32, kind="ExternalInput").ap()
    wc_in = nc.dram_tensor("w_context", [E], F32, kind="ExternalInput").ap()
    wm_in = nc.dram_tensor("w_multiple", [E], F32, kind="ExternalInput").ap()
    out_ext = nc.dram_tensor("out", [BPC, C, 4 * E], F32, kind="ExternalOutput").ap()
    with tile.TileContext(nc) as tc:
        _body(tc, out_ext, ctx_in, q_in, wq_in, wc_in, wm_in)
    nc.compile()
    _NC_CACHE = nc
    return nc


def _run(inputs, trace=False, **kw):
    nc = _build()
    context = np.ascontiguousarray(np.asarray(inputs["context"], dtype=np.float32))
    question = np.ascontiguousarray(np.asarray(inputs["question"], dtype=np.float32))
    wq = np.ascontiguousarray(np.asarray(inputs["w_question"], dtype=np.float32))
    wc = np.ascontiguousarray(np.asarray(inputs["w_context"], dtype=np.float32))
    wm = np.ascontiguousarray(np.asarray(inputs["w_multiple"], dtype=np.float32))
    in_maps = []
    for i in range(NCORES):
        sl = slice(i * BPC, (i + 1) * BPC)
        in_maps.append(
            {
                "context": context[sl],
                "question": question[sl],
                "w_question": wq,
                "w_context": wc,
                "w_multiple": wm,
            }
        )
    res = run_bass_kernel_spmd(
        nc, in_maps, core_ids=list(range(NCORES)), trace=trace, **kw
    )
    out = np.concatenate([res.results[i]["out"] for i in range(NCORES)], axis=0)
    return out, res


def kernel(**inputs):
    out, _ = _run(inputs, trace=False)
    return out


# revision 24
# speedup vs baseline: 1.0539x; 1.0539x over previous
"""Trainium2 Bass kernel for BiDAF-style bidirectional attention.

Reference math (per batch b):
    sim[c,q]  = q[q]·wq + c[c]·wc + sum_e wm[e]*question[q,e]*context[c,e]
    c2q[c,:]  = softmax_q(sim[c,:]) @ question          # (C, E)
    q2c[:]    = softmax_c(max_q sim[c,:]) @ context     # (E,)
    out[c,:]  = [context | c2q | context*c2q | context*q2c]

Sharding: pure data parallel over batch (B=16 -> 2 batches per core x 8 cores).

Context tiles (128 rows of C on partitions) are processed in PAIRS so the
small vector-engine ops amortize their fixed per-op cost:
  - PE transposes Xc (fp32) -> XcT psum; the PSUM->SBUF copy rounds to fp32r
  - sim psum (128, 2, 256) accumulates XcT.T @ rhs_aug per tile (fp32r,
    N=256 to hit the single-pass fast path; col 128 carries wc·ctx for the
    q2c stats, cols 129:255 are zero pad)
  - one DVE add applies the q-weighted row to both tiles; 3D-AP reduce_max
    gets both row-maxes in one op
  - exp on the scalar engine emits bf16 attention weights + fp32 row-sums
  - attn^T (bf16 PE transpose) @ question (bf16) -> c2q, row-rescaled by
    1/rowsum during the PSUM->SBUF copy on the scalar engine
  - q2c (second pass, softmax over C of the row maxes): global max/sum via
    PE-transpose + ones-matmul broadcasts; 16 rank-1 fp32 matmuls
  - outputs assemble in a (128, 1024) staging tile per C-tile; cols 0:768
    ship in pass 1, 768:1024 in pass 2 (wide DMA bursts); the elementwise
    products run on GpSimd to keep the vector engine free
"""

import numpy as np

import concourse.bass as bass
import concourse.tile as tile
import concourse.mybir as mybir
from concourse import bacc
from concourse.bass_utils import run_bass_kernel_spmd
from concourse.masks import make_identity

B, C, Q, E = 16, 2048, 128, 256
NCORES = 8
BPC = B // NCORES          # batches per core
NT = C // 128              # context tiles per batch
F32 = mybir.dt.float32
F32R = mybir.dt.float32r
BF16 = mybir.dt.bfloat16


def _body(tc, out_ext, ctx_in, q_in, wq_in, wc_in, wm_in):
    nc = tc.nc
    with (
        tc.tile_pool(name="singles", bufs=1) as singles,
        tc.tile_pool(name="xcpool", bufs=9) as xcp,
        tc.tile_pool(name="qside", bufs=2) as qside,
        tc.tile_pool(name="xbfpool", bufs=9) as xbfp,
        tc.tile_pool(name="work", bufs=4) as work,
        tc.tile_pool(name="statsp", bufs=2) as statsp,
        tc.tile_pool(name="ps_xct", bufs=2, space="PSUM") as ps_xct,
        tc.tile_pool(name="ps_sim", bufs=2, space="PSUM") as ps_sim,
        tc.tile_pool(name="ps_pt", bufs=1, space="PSUM") as ps_pt,
        tc.tile_pool(name="ps_c2q", bufs=1, space="PSUM") as ps_c2q,
        tc.tile_pool(name="ps_q2c", bufs=1, space="PSUM") as ps_q2c,
        tc.tile_pool(name="ps_misc", bufs=1, space="PSUM") as ps_misc,
    ):
        ident = singles.tile([128, 128], F32)
        make_identity(nc, ident)
        ident_bf = singles.tile([128, 128], BF16)
        make_identity(nc, ident_bf)
        ones_r = singles.tile([1, 128], F32)
        nc.vector.memset(ones_r, 1.0)
        ones_c = singles.tile([128, 1], F32)
        nc.vector.memset(ones_c, 1.0)
        # rank-1 params laid out (128 partitions, 2 chunks of E)
        wq_sb = singles.tile([128, 2], F32)
        nc.sync.dma_start(out=wq_sb, in_=wq_in.rearrange("(j p) -> p j", p=128))
        wc_sb = singles.tile([128, 2], F32)
        nc.sync.dma_start(out=wc_sb, in_=wc_in.rearrange("(j p) -> p j", p=128))
        wm_sb = singles.tile([128, 2], F32)
        nc.sync.dma_start(out=wm_sb, in_=wm_in.rearrange("(j p) -> p j", p=128))

        mstats = {}
        groups_all = {}
        for b in range(BPC):
            # ---- phase A: question-side prep -------------------------------
            qm = qside.tile([128, E], F32)
            nc.sync.dma_start(out=qm, in_=q_in[b])
            qmt_ps = ps_xct.tile([128, E], F32, tag="xct")
            for j in range(2):
                nc.tensor.transpose(
                    qmt_ps[:, j * 128 : (j + 1) * 128],
                    qm[:, j * 128 : (j + 1) * 128],
                    ident,
                )
            qmt_sb = qside.tile([128, E], F32)
            nc.vector.tensor_copy(out=qmt_sb, in_=qmt_ps)
            # bf16 copy of the question for the c2q matmul
            qm_bf = qside.tile([128, E], BF16)
            nc.vector.tensor_copy(out=qm_bf, in_=qm)
            # rhs_aug[:, j, 0:128] = wm-chunk * QmT-chunk ; [:, j, 128] = wc-chunk
            # cols 129:256 are zero padding so the fp32r matmul runs at N=256.
            rhs_aug = qside.tile([128, 2, E], F32R)
            for j in range(2):
                nc.vector.tensor_scalar_mul(
                    rhs_aug[:, j, 0:128],
                    qmt_sb[:, j * 128 : (j + 1) * 128],
                    wm_sb[:, j : j + 1],
                )
                nc.vector.tensor_copy(
                    out=rhs_aug[:, j, 128:129], in_=wc_sb[:, j : j + 1]
                )
                # memset can't write f32r; x0.0 DVE op produces rounded zeros
                nc.vector.tensor_scalar_mul(
                    rhs_aug[:, j, 129:256],
                    qmt_sb[:, j * 128 : (j + 1) * 128][:, 0:127],
                    0.0,
                )
            # q_weighted row (1, Q) -> broadcast to 128 partitions, twice wide
            qw_ps = ps_misc.tile([1, 128], F32, tag="misc")
            for j in range(2):
                nc.tensor.matmul(
                    qw_ps,
                    wq_sb[:, j : j + 1],
                    qmt_sb[:, j * 128 : (j + 1) * 128],
                    start=(j == 0),
                    stop=(j == 1),
                )
            qw_row = qside.tile([1, 128], F32)
            nc.vector.tensor_copy(out=qw_row, in_=qw_ps)
            qwb_ps = ps_misc.tile([128, 128], F32, tag="misc")
            nc.tensor.matmul(qwb_ps, ones_r, qw_row, start=True, stop=True)
            qw_bcast2 = qside.tile([128, 2, 128], F32)
            nc.vector.tensor_copy(out=qw_bcast2[:, 0, :], in_=qwb_ps)
            nc.vector.tensor_copy(out=qw_bcast2[:, 1, :], in_=qwb_ps)

            mstat = statsp.tile([128, NT], F32)
            mstats[b] = mstat
            groups = []
            groups_all[b] = groups

            # ---- phase B: pass 1, groups of 4 context tiles ----------------
            for g in range(NT // 4):
                stg = xcp.tile([128, 4, 4 * E], F32, tag="stg")
                groups.append(stg)
                nc.sync.dma_start(
                    out=stg[:, :, 0:E],
                    in_=ctx_in[b, g * 512 : (g + 1) * 512, :].rearrange(
                        "(t p) e -> p t e", p=128
                    ),
                )
                for h in range(2):
                    sim_ps = ps_sim.tile([128, 2, E], F32, tag="sim")
                    for i in range(2):
                        lane = 2 * h + i
                        xc = stg[:, lane, 0:E]
                        xct_ps = ps_xct.tile([128, E], F32, tag="xct")
                        for j in range(2):
                            nc.tensor.transpose(
                                xct_ps[:, j * 128 : (j + 1) * 128],
                                xc[:, j * 128 : (j + 1) * 128],
                                ident,
                            )
                        xct_sb = work.tile([128, E], F32R, tag="xct_sb")
                        nc.vector.tensor_copy(out=xct_sb, in_=xct_ps)
                        for j in range(2):
                            nc.tensor.matmul(
                                sim_ps[:, i, :],
                                xct_sb[:, j * 128 : (j + 1) * 128],
                                rhs_aug[:, j, :],
                                start=(j == 0),
                                stop=(j == 1),
                            )

                    # add the q-weighted row for both tiles in one DVE op
                    sim_in = work.tile([128, 2, 128], F32, tag="sim_in")
                    nc.vector.tensor_add(sim_in, sim_ps[:, :, 0:128], qw_bcast2)
                    neg_m = work.tile([128, 2], F32, tag="neg_m")
                    nc.vector.reduce_max(
                        out=neg_m,
                        in_=sim_in,
                        axis=mybir.AxisListType.X,
                        negate=True,
                    )
                    # q2c stats: max_q(full sim) = wc·ctx (col 128) + row-max
                    t0 = 4 * g + 2 * h
                    nc.vector.tensor_sub(
                        mstat[:, t0 : t0 + 2], sim_ps[:, :, 128], neg_m
                    )
                    row_sum = work.tile([128, 2], F32, tag="row_sum")
                    recip = work.tile([128, 2], F32, tag="recip")
                    p_sb = work.tile([128, 2, 128], BF16, tag="p_sb")
                    for i in range(2):
                        nc.scalar.activation(
                            out=p_sb[:, i, :],
                            in_=sim_in[:, i, :],
                            func=mybir.ActivationFunctionType.Exp,
                            bias=neg_m[:, i : i + 1],
                            scale=1.0,
                            accum_out=row_sum[:, i : i + 1],
                        )
                    nc.vector.reciprocal(out=recip, in_=row_sum)

                    for i in range(2):
                        lane = 2 * h + i
                        pt_ps = ps_pt.tile([128, 128], BF16, tag="pt")
                        nc.tensor.transpose(pt_ps, p_sb[:, i, :], ident_bf)
                        pt_sb = work.tile([128, 128], BF16, tag="pt_sb")
                        nc.vector.tensor_copy(out=pt_sb, in_=pt_ps)

                        c2q_ps = ps_c2q.tile([128, E], F32, tag="c2q")
                        nc.tensor.matmul(
                            c2q_ps, pt_sb, qm_bf, start=True, stop=True
                        )
                        nc.scalar.activation(
                            out=stg[:, lane, E : 2 * E],
                            in_=c2q_ps,
                            func=mybir.ActivationFunctionType.Copy,
                            scale=recip[:, i : i + 1],
                        )

                # ctx * c2q for all 4 lanes in one op, then one wide store
                nc.gpsimd.tensor_mul(
                    stg[:, :, 2 * E : 3 * E],
                    stg[:, :, 0:E],
                    stg[:, :, E : 2 * E],
                )
                nc.sync.dma_start(
                    out=out_ext[b, g * 512 : (g + 1) * 512, 0 : 3 * E].rearrange(
                        "(t p) f -> p t f", p=128
                    ),
                    in_=stg[:, :, 0 : 3 * E],
                )

        for b in range(BPC):
            mstat = mstats[b]
            groups = groups_all[b]
            # ---- phase C: q2c attention over the context axis --------------
            r1 = statsp.tile([128, 1], F32, tag="r1")
            nc.vector.reduce_max(out=r1, in_=mstat, axis=mybir.AxisListType.X)
            r1t_ps = ps_misc.tile([1, 128], F32, tag="misc")
            nc.tensor.transpose(r1t_ps, r1, ident)
            neg_gmax = statsp.tile([1, 1], F32, tag="neg_gmax")
            nc.vector.reduce_max(
                out=neg_gmax, in_=r1t_ps, axis=mybir.AxisListType.X, negate=True
            )
            ngb_ps = ps_misc.tile([128, 1], F32, tag="misc")
            nc.tensor.matmul(ngb_ps, ones_r, neg_gmax, start=True, stop=True)
            ngb_sb = statsp.tile([128, 1], F32, tag="ngb_sb")
            nc.vector.tensor_copy(out=ngb_sb, in_=ngb_ps)

            e_sb = statsp.tile([128, NT], BF16, tag="e_sb")
            s_col = statsp.tile([128, 1], F32, tag="s_col")
            nc.scalar.activation(
                out=e_sb,
                in_=mstat,
                func=mybir.ActivationFunctionType.Exp,
                bias=ngb_sb,
                scale=1.0,
                accum_out=s_col,
            )
            tot_ps = ps_misc.tile([1, 1], F32, tag="misc")
            nc.tensor.matmul(tot_ps, s_col, ones_c, start=True, stop=True)
            rt_sb = statsp.tile([1, 1], F32, tag="rt_sb")
            nc.vector.reciprocal(out=rt_sb, in_=tot_ps)

            # bf16 context copies (cold-clock bf16 matmuls are 4x faster
            # than fp32 ones); alternate engines so the casts parallelize
            xcbfs = []
            for g in range(NT // 4):
                xcbf = xbfp.tile([128, 4, E], BF16, tag="xcbf")
                xcbfs.append(xcbf)
                eng = (
                    nc.gpsimd if (b == BPC - 1 and g % 2 == 1) else nc.vector
                )
                eng.tensor_copy(out=xcbf, in_=groups[g][:, :, 0:E])
            q2c_ps = ps_q2c.tile([1, E], F32, tag="q2c")
            for t in range(NT):
                nc.tensor.matmul(
                    q2c_ps,
                    e_sb[:, t : t + 1],
                    xcbfs[t // 4][:, t % 4, :],
                    start=(t == 0),
                    stop=(t == NT - 1),
                )
            q2c_sb = statsp.tile([1, E], F32, tag="q2c_sb")
            nc.scalar.activation(
                out=q2c_sb,
                in_=q2c_ps,
                func=mybir.ActivationFunctionType.Copy,
                scale=rt_sb,
            )
            q2cb_ps = ps_misc.tile([128, E], F32, tag="misc")
            nc.tensor.matmul(q2cb_ps, ones_r, q2c_sb, start=True, stop=True)
            q2cb_sb = statsp.tile([128, 4, E], F32, tag="q2cb_sb")
            for lane in range(4):
                nc.vector.tensor_copy(out=q2cb_sb[:, lane, :], in_=q2cb_ps)

            # ---- phase D: pass 2 -> context * q2c ---------------------------
            for g in range(NT // 4):
                stg = groups[g]
                eng = (
                    nc.gpsimd if (b == BPC - 1 and g % 2 == 1) else nc.vector
                )
                eng.tensor_mul(
                    stg[:, :, 3 * E : 4 * E], stg[:, :, 0:E], q2cb_sb
                )
                nc.sync.dma_start(
                    out=out_ext[
                        b, g * 512 : (g + 1) * 512, 3 * E : 4 * E
                    ].rearrange("(t p) f -> p t f", p=128),
                    in_=stg[:, :, 3 * E : 4 * E],
                )


_NC_CACHE = None


def _build():
    global _NC_CACHE
    if _NC_CACHE is not None:
        return _NC_CACHE
    nc = bacc.Bacc(
        "TRN2", target_bir_lowering=False, debug=False, num_devices=NCORES
    )
    ctx_in = nc.dram_tensor("context", [BPC, C, E], F32, kind="ExternalInput").ap()
    q_in = nc.dram_tensor("question", [BPC, Q, E], F32, kind="ExternalInput").ap()
    wq_in = nc.dram_tensor("w_question", [E], F32, kind="ExternalInput").ap()
    wc_in = nc.dram_tensor("w_context", [E], F32, kind="ExternalInput").ap()
    wm_in = nc.dram_tensor("w_multiple", [E], F32, kind="ExternalInput").ap()
    out_ext = nc.dram_tensor("out", [BPC, C, 4 * E], F32, kind="ExternalOutput").ap()
    with tile.TileContext(nc) as tc:
        _body(tc, out_ext, ctx_in, q_in, wq_in, wc_in, wm_in)
    nc.compile()
    _NC_CACHE = nc
    return nc


def _run(inputs, trace=False, **kw):
    nc = _build()
    context = np.ascontiguousarray(np.asarray(inputs["context"], dtype=np.float32))
    question = np.ascontiguousarray(np.asarray(inputs["question"], dtype=np.float32))
    wq = np.ascontiguousarray(np.asarray(inputs["w_question"], dtype=np.float32))
    wc = np.ascontiguousarray(np.asarray(inputs["w_context"], dtype=np.float32))
    wm = np.ascontiguousarray(np.asarray(inputs["w_multiple"], dtype=np.float32))
    in_maps = []
    for i in range(NCORES):
        sl = slice(i * BPC, (i + 1) * BPC)
        in_maps.append(
            {
                "context": context[sl],
                "question": question[sl],
                "w_question": wq,
                "w_context": wc,
                "w_multiple": wm,
            }
        )
    res = run_bass_kernel_spmd(
        nc, in_maps, core_ids=list(range(NCORES)), trace=trace, **kw
    )
    out = np.concatenate([res.results[i]["out"] for i in range(NCORES)], axis=0)
    return out, res


def kernel(**inputs):
    out, _ = _run(inputs, trace=False)
    return out


# revision 25
# speedup vs baseline: 1.1587x; 1.0994x over previous
"""Trainium2 Bass kernel for BiDAF-style bidirectional attention.

Reference math (per batch b):
    sim[c,q]  = q[q]·wq + c[c]·wc + sum_e wm[e]*question[q,e]*context[c,e]
    c2q[c,:]  = softmax_q(sim[c,:]) @ question          # (C, E)
    q2c[:]    = softmax_c(max_q sim[c,:]) @ context     # (E,)
    out[c,:]  = [context | c2q | context*c2q | context*q2c]

Sharding: pure data parallel over batch (B=16 -> 2 batches per core x 8 cores).

Context tiles (128 rows of C on partitions) are processed in PAIRS so the
small vector-engine ops amortize their fixed per-op cost:
  - PE transposes Xc (fp32) -> XcT psum; the PSUM->SBUF copy rounds to fp32r
  - sim psum (128, 2, 256) accumulates XcT.T @ rhs_aug per tile (fp32r,
    N=256 to hit the single-pass fast path; col 128 carries wc·ctx for the
    q2c stats, cols 129:255 are zero pad)
  - one DVE add applies the q-weighted row to both tiles; 3D-AP reduce_max
    gets both row-maxes in one op
  - exp on the scalar engine emits bf16 attention weights + fp32 row-sums
  - attn^T (bf16 PE transpose) @ question (bf16) -> c2q, row-rescaled by
    1/rowsum during the PSUM->SBUF copy on the scalar engine
  - q2c (second pass, softmax over C of the row maxes): global max/sum via
    PE-transpose + ones-matmul broadcasts; 16 rank-1 fp32 matmuls
  - outputs assemble in a (128, 1024) staging tile per C-tile; cols 0:768
    ship in pass 1, 768:1024 in pass 2 (wide DMA bursts); the elementwise
    products run on GpSimd to keep the vector engine free
"""

import numpy as np

import concourse.bass as bass
import concourse.tile as tile
import concourse.mybir as mybir
from concourse import bacc
from concourse.bass_utils import run_bass_kernel_spmd
from concourse.masks import make_identity

B, C, Q, E = 16, 2048, 128, 256
NCORES = 8
BPC = B // NCORES          # batches per core
NT = C // 128              # context tiles per batch
F32 = mybir.dt.float32
F32R = mybir.dt.float32r
BF16 = mybir.dt.bfloat16


def _body(tc, out_ext, ctx_in, q_in, wq_in, wc_in, wm_in):
    nc = tc.nc
    with (
        tc.tile_pool(name="singles", bufs=1) as singles,
        tc.tile_pool(name="xcpool", bufs=9) as xcp,
        tc.tile_pool(name="qside", bufs=2) as qside,
        tc.tile_pool(name="xbfpool", bufs=9) as xbfp,
        tc.tile_pool(name="work", bufs=4) as work,
        tc.tile_pool(name="statsp", bufs=2) as statsp,
        tc.tile_pool(name="ps_xct", bufs=2, space="PSUM") as ps_xct,
        tc.tile_pool(name="ps_sim", bufs=2, space="PSUM") as ps_sim,
        tc.tile_pool(name="ps_pt", bufs=1, space="PSUM") as ps_pt,
        tc.tile_pool(name="ps_c2q", bufs=1, space="PSUM") as ps_c2q,
        tc.tile_pool(name="ps_q2c", bufs=1, space="PSUM") as ps_q2c,
        tc.tile_pool(name="ps_misc", bufs=1, space="PSUM") as ps_misc,
    ):
        ident = singles.tile([128, 128], F32)
        make_identity(nc, ident)
        ident_bf = singles.tile([128, 128], BF16)
        make_identity(nc, ident_bf)
        ones_r = singles.tile([1, 128], F32)
        nc.vector.memset(ones_r, 1.0)
        ones_c = singles.tile([128, 1], F32)
        nc.vector.memset(ones_c, 1.0)
        # rank-1 params laid out (128 partitions, 2 chunks of E)
        wq_sb = singles.tile([128, 2], F32)
        nc.sync.dma_start(out=wq_sb, in_=wq_in.rearrange("(j p) -> p j", p=128))
        wc_sb = singles.tile([128, 2], F32)
        nc.sync.dma_start(out=wc_sb, in_=wc_in.rearrange("(j p) -> p j", p=128))
        wm_sb = singles.tile([128, 2], F32)
        nc.sync.dma_start(out=wm_sb, in_=wm_in.rearrange("(j p) -> p j", p=128))

        mstats = {}
        groups_all = {}
        for b in range(BPC):
            # ---- phase A: question-side prep -------------------------------
            qm = qside.tile([128, E], F32)
            nc.sync.dma_start(out=qm, in_=q_in[b])
            qmt_ps = ps_xct.tile([128, E], F32, tag="xct")
            for j in range(2):
                nc.tensor.transpose(
                    qmt_ps[:, j * 128 : (j + 1) * 128],
                    qm[:, j * 128 : (j + 1) * 128],
                    ident,
                )
            qmt_sb = qside.tile([128, E], F32)
            nc.vector.tensor_copy(out=qmt_sb, in_=qmt_ps)
            # bf16 copy of the question for the c2q matmul
            qm_bf = qside.tile([128, E], BF16)
            nc.vector.tensor_copy(out=qm_bf, in_=qm)
            # rhs_aug[:, j, 0:128] = wm-chunk * QmT-chunk ; [:, j, 128] = wc-chunk
            # cols 129:256 are zero padding so the fp32r matmul runs at N=256.
            rhs_aug = qside.tile([128, 2, E], F32R)
            for j in range(2):
                nc.vector.tensor_scalar_mul(
                    rhs_aug[:, j, 0:128],
                    qmt_sb[:, j * 128 : (j + 1) * 128],
                    wm_sb[:, j : j + 1],
                )
                nc.vector.tensor_copy(
                    out=rhs_aug[:, j, 128:129], in_=wc_sb[:, j : j + 1]
                )
                # memset can't write f32r; x0.0 DVE op produces rounded zeros
                nc.vector.tensor_scalar_mul(
                    rhs_aug[:, j, 129:256],
                    qmt_sb[:, j * 128 : (j + 1) * 128][:, 0:127],
                    0.0,
                )
            # q_weighted row (1, Q) -> broadcast to 128 partitions, twice wide
            qw_ps = ps_misc.tile([1, 128], F32, tag="misc")
            for j in range(2):
                nc.tensor.matmul(
                    qw_ps,
                    wq_sb[:, j : j + 1],
                    qmt_sb[:, j * 128 : (j + 1) * 128],
                    start=(j == 0),
                    stop=(j == 1),
                )
            qw_row = qside.tile([1, 128], F32)
            nc.vector.tensor_copy(out=qw_row, in_=qw_ps)
            qwb_ps = ps_misc.tile([128, 128], F32, tag="misc")
            nc.tensor.matmul(qwb_ps, ones_r, qw_row, start=True, stop=True)
            qw_bcast2 = qside.tile([128, 2, 128], F32)
            nc.vector.tensor_copy(out=qw_bcast2[:, 0, :], in_=qwb_ps)
            nc.vector.tensor_copy(out=qw_bcast2[:, 1, :], in_=qwb_ps)

            mstat = statsp.tile([128, NT], F32)
            mstats[b] = mstat
            groups = []
            groups_all[b] = groups

            # ---- phase B: pass 1, groups of 4 context tiles ----------------
            for g in range(NT // 4):
                stg = xcp.tile([128, 4, 4 * E], F32, tag="stg")
                groups.append(stg)
                nc.sync.dma_start(
                    out=stg[:, :, 0:E],
                    in_=ctx_in[b, g * 512 : (g + 1) * 512, :].rearrange(
                        "(t p) e -> p t e", p=128
                    ),
                )
                for h in range(2):
                    sim_ps = ps_sim.tile([128, 2, E], F32, tag="sim")
                    for i in range(2):
                        lane = 2 * h + i
                        xc = stg[:, lane, 0:E]
                        xct_ps = ps_xct.tile([128, E], F32, tag="xct")
                        for j in range(2):
                            nc.tensor.transpose(
                                xct_ps[:, j * 128 : (j + 1) * 128],
                                xc[:, j * 128 : (j + 1) * 128],
                                ident,
                            )
                        xct_sb = work.tile([128, E], F32R, tag="xct_sb")
                        nc.vector.tensor_copy(out=xct_sb, in_=xct_ps)
                        for j in range(2):
                            nc.tensor.matmul(
                                sim_ps[:, i, :],
                                xct_sb[:, j * 128 : (j + 1) * 128],
                                rhs_aug[:, j, :],
                                start=(j == 0),
                                stop=(j == 1),
                            )

                    # add the q-weighted row for both tiles in one DVE op
                    sim_in = work.tile([128, 2, 128], F32, tag="sim_in")
                    nc.vector.tensor_add(sim_in, sim_ps[:, :, 0:128], qw_bcast2)
                    neg_m = work.tile([128, 2], F32, tag="neg_m")
                    nc.vector.reduce_max(
                        out=neg_m,
                        in_=sim_in,
                        axis=mybir.AxisListType.X,
                        negate=True,
                    )
                    # q2c stats: max_q(full sim) = wc·ctx (col 128) + row-max
                    t0 = 4 * g + 2 * h
                    nc.vector.tensor_sub(
                        mstat[:, t0 : t0 + 2], sim_ps[:, :, 128], neg_m
                    )
                    row_sum = work.tile([128, 2], F32, tag="row_sum")
                    recip = work.tile([128, 2], F32, tag="recip")
                    p_sb = work.tile([128, 2, 128], BF16, tag="p_sb")
                    for i in range(2):
                        nc.scalar.activation(
                            out=p_sb[:, i, :],
                            in_=sim_in[:, i, :],
                            func=mybir.ActivationFunctionType.Exp,
                            bias=neg_m[:, i : i + 1],
                            scale=1.0,
                            accum_out=row_sum[:, i : i + 1],
                        )
                    nc.vector.reciprocal(out=recip, in_=row_sum)

                    for i in range(2):
                        lane = 2 * h + i
                        pt_ps = ps_pt.tile([128, 128], BF16, tag="pt")
                        nc.tensor.transpose(pt_ps, p_sb[:, i, :], ident_bf)
                        pt_sb = work.tile([128, 128], BF16, tag="pt_sb")
                        nc.vector.tensor_copy(out=pt_sb, in_=pt_ps)

                        c2q_ps = ps_c2q.tile([128, E], F32, tag="c2q")
                        nc.tensor.matmul(
                            c2q_ps, pt_sb, qm_bf, start=True, stop=True
                        )
                        nc.scalar.activation(
                            out=stg[:, lane, E : 2 * E],
                            in_=c2q_ps,
                            func=mybir.ActivationFunctionType.Copy,
                            scale=recip[:, i : i + 1],
                        )

                # ctx * c2q for all 4 lanes in one op, then one wide store
                nc.gpsimd.tensor_mul(
                    stg[:, :, 2 * E : 3 * E],
                    stg[:, :, 0:E],
                    stg[:, :, E : 2 * E],
                )
                nc.sync.dma_start(
                    out=out_ext[b, g * 512 : (g + 1) * 512, 0 : 3 * E].rearrange(
                        "(t p) f -> p t f", p=128
                    ),
                    in_=stg[:, :, 0 : 3 * E],
                )

        for b in range(BPC):
            mstat = mstats[b]
            groups = groups_all[b]
            # ---- phase C: q2c attention over the context axis --------------
            r1 = statsp.tile([128, 1], F32, tag="r1")
            nc.vector.reduce_max(out=r1, in_=mstat, axis=mybir.AxisListType.X)
            r1t_ps = ps_misc.tile([1, 128], F32, tag="misc")
            nc.tensor.transpose(r1t_ps, r1, ident)
            neg_gmax = statsp.tile([1, 1], F32, tag="neg_gmax")
            nc.vector.reduce_max(
                out=neg_gmax, in_=r1t_ps, axis=mybir.AxisListType.X, negate=True
            )
            ngb_ps = ps_misc.tile([128, 1], F32, tag="misc")
            nc.tensor.matmul(ngb_ps, ones_r, neg_gmax, start=True, stop=True)
            ngb_sb = statsp.tile([128, 1], F32, tag="ngb_sb")
            nc.vector.tensor_copy(out=ngb_sb, in_=ngb_ps)

            e_sb = statsp.tile([128, NT], BF16, tag="e_sb")
            s_col = statsp.tile([128, 1], F32, tag="s_col")
            nc.scalar.activation(
                out=e_sb,
                in_=mstat,
                func=mybir.ActivationFunctionType.Exp,
                bias=ngb_sb,
                scale=1.0,
                accum_out=s_col,
            )
            tot_ps = ps_misc.tile([1, 1], F32, tag="misc")
            nc.tensor.matmul(tot_ps, s_col, ones_c, start=True, stop=True)
            rt_sb = statsp.tile([1, 1], F32, tag="rt_sb")
            nc.vector.reciprocal(out=rt_sb, in_=tot_ps)

            # bf16 context copies (cold-clock bf16 matmuls are 4x faster
            # than fp32 ones); alternate engines so the casts parallelize
            xcbfs = []
            for g in range(NT // 4):
                xcbf = xbfp.tile([128, 4, E], BF16, tag="xcbf")
                xcbfs.append(xcbf)
                nc.vector.tensor_copy(out=xcbf, in_=groups[g][:, :, 0:E])
            q2c_ps = ps_q2c.tile([1, E], F32, tag="q2c")
            for t in range(NT):
                nc.tensor.matmul(
                    q2c_ps,
                    e_sb[:, t : t + 1],
                    xcbfs[t // 4][:, t % 4, :],
                    start=(t == 0),
                    stop=(t == NT - 1),
                )
            q2c_sb = statsp.tile([1, E], F32, tag="q2c_sb")
            nc.scalar.activation(
                out=q2c_sb,
                in_=q2c_ps,
                func=mybir.ActivationFunctionType.Copy,
                scale=rt_sb,
            )
            q2cb_ps = ps_misc.tile([128, E], F32, tag="misc")
            nc.tensor.matmul(q2cb_ps, ones_r, q2c_sb, start=True, stop=True)
            q2cb_sb = statsp.tile([128, 4, E], F32, tag="q2cb_sb")
            for lane in range(4):
                nc.vector.tensor_copy(out=q2cb_sb[:, lane, :], in_=q2cb_ps)

            # ---- phase D: pass 2 -> context * q2c ---------------------------
            for g in range(NT // 4):
                stg = groups[g]
                nc.vector.tensor_mul(
                    stg[:, :, 3 * E : 4 * E], stg[:, :, 0:E], q2cb_sb
                )
                nc.sync.dma_start(
                    out=out_ext[
                        b, g * 512 : (g + 1) * 512, 3 * E : 4 * E
                    ].rearrange("(t p) f -> p t f", p=128),
                    in_=stg[:, :, 3 * E : 4 * E],
                )


_NC_CACHE = None


def _build():
    global _NC_CACHE
    if _NC_CACHE is not None:
        return _NC_CACHE
    nc = bacc.Bacc(
        "TRN2", target_bir_lowering=False, debug=False, num_devices=NCORES
    )
    ctx_in = nc.dram_tensor("context", [BPC, C, E], F32, kind="ExternalInput").ap()
    q_in = nc.dram_tensor("question", [BPC, Q, E], F32, kind="ExternalInput").ap()
    wq_in = nc.dram_tensor("w_question", [E], F32, kind="ExternalInput").ap()
    wc_in = nc.dram_tensor("w_context", [E], F32, kind="ExternalInput").ap()
    wm_in = nc.dram_tensor("w_multiple", [E], F32, kind="ExternalInput").ap()
    out_ext = nc.dram_tensor("out", [BPC, C, 4 * E], F32, kind="ExternalOutput").ap()
    with tile.TileContext(nc) as tc:
        _body(tc, out_ext, ctx_in, q_in, wq_in, wc_in, wm_in)
    nc.compile()
    _NC_CACHE = nc
    return nc


def _run(inputs, trace=False, **kw):
    nc = _build()
    context = np.ascontiguousarray(np.asarray(inputs["context"], dtype=np.float32))
    question = np.ascontiguousarray(np.asarray(inputs["question"], dtype=np.float32))
    wq = np.ascontiguousarray(np.asarray(inputs["w_question"], dtype=np.float32))
    wc = np.ascontiguousarray(np.asarray(inputs["w_context"], dtype=np.float32))
    wm = np.ascontiguousarray(np.asarray(inputs["w_multiple"], dtype=np.float32))
    in_maps = []
    for i in range(NCORES):
        sl = slice(i * BPC, (i + 1) * BPC)
        in_maps.append(
            {
                "context": context[sl],
                "question": question[sl],
                "w_question": wq,
                "w_context": wc,
                "w_multiple": wm,
            }
        )
    res = run_bass_kernel_spmd(
        nc, in_maps, core_ids=list(range(NCORES)), trace=trace, **kw
    )
    out = np.concatenate([res.results[i]["out"] for i in range(NCORES)], axis=0)
    return out, res


def kernel(**inputs):
    out, _ = _run(inputs, trace=False)
    return out
